# revision 3
# baseline (speedup 1.0000x reference)
"""GAT (2-layer, 4-head) on 8 Trainium2 NeuronCores.

Strategy (dst-sharded, per sharding hint):
  - Nodes partitioned into 8 contiguous blocks of 6250; core c owns block c
    and all edges whose dst lies in it.
  - Host bin-packs each core's dst nodes into 200 "windows" of <=32 dsts,
    bounding per-window lo/hi edge counts (lo = src owner < 5, needed because
    dma_gather indices are signed int16).  Nodes are stored in permuted
    (window) order on device; the host un-permutes the final output.
  - Per layer: feat = h @ W computed locally (PE), attention projections
    el/er via tiny matmuls, feature table AllGather'ed, then per-edge rows
    gathered with dma_gather; edge softmax + weighted segment-sum are done
    as one-hot matmuls on the PE with masks precomputed on host.
"""

import sys

sys.path.insert(0, "/opt/trn_rl_repo")

import numpy as np

import concourse.bacc as bacc
import concourse.mybir as mybir
import concourse.tile as tile
from concourse import bass
from concourse.bass_utils import run_bass_kernel_spmd
from concourse.masks import make_identity

# ---------------- problem constants ----------------
N = 50000
E = 800000
D = 256           # in feats = H*F
H = 4
F = 64
NC = 8
NLOC = N // NC    # 6250
NEG_SLOPE = 0.2

# ---------------- sharding constants ----------------
WSPAN = 32        # dsts per window
NWIN = 200        # windows per core
C_LO = 3          # lo chunks per window (<=384 lo edges)
C_HI = 2          # hi chunks per window (<=256 hi edges)
C_W = C_LO + C_HI
WG = 8            # windows per group
NGRP = NWIN // WG          # 25
CH_PER_G = WG * C_W        # 40 chunks per group
CALLS_PER_G = CH_PER_G // 8  # 5 gather calls per group (1024 idx each)
NCH = NWIN * C_W           # 1000 chunks per layer
SLOTS = NCH * 128          # 128000 edge slots
NPAD = NWIN * WSPAN        # 6400 padded rows per core
RW = 320          # table row width (f32): 256 feat | 1 one | 4 el | 59 pad
ERW = 64          # er table row width (256B rows)
LO_ROWS = 5 * NPAD          # 32000 (int16-safe)
NTILES = NPAD // 128        # 50
NEG_BIG = -1.0e30

_PROGRAM_CACHE = {}


# =====================================================================
# host-side graph preprocessing
# =====================================================================

def _preprocess_core(src_c, dst_loc_c):
    """Pack one core's edges into windows/chunks/slots.

    Returns (perm, gidx, eidx, mask) where
      perm[i]   = local node id stored at permuted position i (len NPAD, -1 pad)
      gidx      = [128, SLOTS//16] int16 gather indices (wrapped, replicated)
      eidx      = same layout, indices into the local (permuted) er table
      mask      = [128, NCH, WSPAN] float32 additive mask
    """
    lo_e = src_c < 5 * NLOC
    # per-node lo/hi degree
    nlo = np.bincount(dst_loc_c[lo_e], minlength=NLOC)
    nhi = np.bincount(dst_loc_c[~lo_e], minlength=NLOC)
    order = np.argsort(-(nlo + nhi), kind="stable")
    cap_lo, cap_hi = C_LO * 128, C_HI * 128
    win_lo = np.zeros(NWIN, np.int64)
    win_hi = np.zeros(NWIN, np.int64)
    win_cnt = np.zeros(NWIN, np.int64)
    node_win = np.full(NLOC, -1, np.int64)
    node_slot = np.full(NLOC, -1, np.int64)
    w_hint = 0
    for u in order:
        placed = False
        for off in range(NWIN):
            w = (w_hint + off) % NWIN
            if (
                win_cnt[w] < WSPAN
                and win_lo[w] + nlo[u] <= cap_lo
                and win_hi[w] + nhi[u] <= cap_hi
            ):
                node_win[u] = w
                node_slot[u] = win_cnt[w]
                win_cnt[w] += 1
                win_lo[w] += nlo[u]
                win_hi[w] += nhi[u]
                w_hint = (w + 1) % NWIN
                placed = True
                break
        if not placed:
            raise RuntimeError("window bin-packing failed")

    perm = np.full(NPAD, -1, np.int64)
    perm_pos = node_win * WSPAN + node_slot          # position of each node
    perm[perm_pos] = np.arange(NLOC)

    # --- assign edges to slots ---
    # sort edges by (window, lo/hi) then lay out sequentially per window
    e_win = node_win[dst_loc_c]
    e_lo = lo_e.astype(np.int64)
    # order: window asc, lo first (lo=1 first -> sort by (win, 1-lo))
    eorder = np.lexsort((1 - e_lo, e_win))
    ew = e_win[eorder]
    el_ = e_lo[eorder]
    esrc = src_c[eorder]
    edst = dst_loc_c[eorder]

    gidx_flat = np.zeros(SLOTS, np.int64)            # gather row id (lo/hi rebased)
    eidx_flat = np.zeros(SLOTS, np.int64)            # permuted local dst row
    mask = np.full((128, NCH, WSPAN), NEG_BIG, np.float32)

    # per-window starting offset of its edges in sorted stream
    win_edge_start = np.zeros(NWIN + 1, np.int64)
    np.add.at(win_edge_start, ew + 1, 1)
    win_edge_start = np.cumsum(win_edge_start)

    owner = esrc // NLOC
    src_loc = esrc - owner * NLOC
    # permuted global id of src (perm of OWNER core needed -> computed later!)
    # placeholder: filled by caller once all cores' perms known.

    slot_of_edge = np.zeros(len(esrc), np.int64)
    for w in range(NWIN):
        s, t = win_edge_start[w], win_edge_start[w + 1]
        if s == t:
            continue
        g, wg = w // WG, w % WG
        nl = int(el_[s:t].sum())
        # lo slots: chunks g*CH_PER_G + wg*C_LO + k
        lo_ch0 = g * CH_PER_G + wg * C_LO
        hi_ch0 = g * CH_PER_G + WG * C_LO + wg * C_HI
        for i in range(t - s):
            if i < nl:
                ch = lo_ch0 + i // 128
                p = i % 128
            else:
                j = i - nl
                ch = hi_ch0 + j // 128
                p = j % 128
            slot = ch * 128 + p
            slot_of_edge[s + i] = slot
            jcol = int(node_slot[edst[s + i]])
            mask[p, ch, jcol] = 0.0
            eidx_flat[slot] = w * WSPAN + jcol

    return dict(
        perm=perm,
        perm_pos=perm_pos,
        slot_of_edge=slot_of_edge,
        esrc=esrc,
        e_lo=el_,
        mask=mask,
        eidx_flat=eidx_flat,
        gidx_flat=gidx_flat,
    )


def _wrap_idx(flat_i64):
    """[SLOTS] -> [128, SLOTS//16] int16 wrapped + replicated."""
    a = flat_i64.astype(np.int16)
    blk = a.reshape(SLOTS // 16, 16).T          # [16, SLOTS//16]
    return np.tile(blk, (8, 1)).copy()


def preprocess(src, dst):
    dst_owner = dst // NLOC
    cores = []
    for c in range(NC):
        m = dst_owner == c
        cores.append(_preprocess_core(src[m], dst[m] - c * NLOC))
    # global permuted ids now that all perms are known
    perm_pos_all = np.stack([cores[c]["perm_pos"] for c in range(NC)])  # [NC, NLOC]
    for c in range(NC):
        cc = cores[c]
        owner = cc["esrc"] // NLOC
        src_loc = cc["esrc"] - owner * NLOC
        gid = owner * NPAD + perm_pos_all[owner, src_loc]
        gl = np.zeros(SLOTS, np.int64)
        gl[cc["slot_of_edge"]] = np.where(cc["e_lo"] == 1, gid, gid - LO_ROWS)
        assert gl.max() < 32768 and gl.min() >= 0
        cc["gidx"] = _wrap_idx(gl)
        cc["eidx"] = _wrap_idx(cc["eidx_flat"])
    return cores


# =====================================================================
# device program
# =====================================================================

def _elu_tile(nc, pool, x, tmp_tag="elu"):
    """In-place-ish ELU on SBUF tile x [128, D]. Returns output tile."""
    m = pool.tile([128, D], mybir.dt.float32, tag=tmp_tag + "_m")
    q = pool.tile([128, D], mybir.dt.float32, tag=tmp_tag + "_q")
    o = pool.tile([128, D], mybir.dt.float32, tag=tmp_tag + "_o")
    nc.vector.tensor_scalar_min(m[:], x[:], 0.0)       # m = min(x,0)
    nc.scalar.activation(q[:], m[:], mybir.ActivationFunctionType.Exp)
    nc.vector.tensor_sub(o[:], x[:], m[:])             # relu(x)
    nc.vector.tensor_add(o[:], o[:], q[:])             # relu(x)+exp(min(x,0))
    nc.vector.tensor_scalar_add(o[:], o[:], -1.0)
    return o


def build_program():
    key = "main"
    if key in _PROGRAM_CACHE:
        return _PROGRAM_CACHE[key]
    nc = bacc.Bacc(
        "TRN2",
        target_bir_lowering=False,
        debug=False,
        num_devices=NC,
        num_swdge_queues=4,
    )
    f32 = mybir.dt.float32
    i16 = mybir.dt.int16

    hperm = nc.dram_tensor("hperm", [NPAD, D], f32, kind="ExternalInput")
    Ws = [nc.dram_tensor(f"W{i}", [D, D], f32, kind="ExternalInput") for i in (1, 2)]
    Wlrs = [nc.dram_tensor(f"Wlr{i}", [D, 2 * H], f32, kind="ExternalInput") for i in (1, 2)]
    gidx_d = nc.dram_tensor("gidx", [128, SLOTS // 16], i16, kind="ExternalInput")
    eidx_d = nc.dram_tensor("eidx", [128, SLOTS // 16], i16, kind="ExternalInput")
    mask_d = nc.dram_tensor("maskd", [128, NCH * WSPAN], f32, kind="ExternalInput")
    out_d = nc.dram_tensor("out", [NPAD, D], f32, kind="ExternalOutput")

    table_loc = nc.dram_tensor("table_loc", [NPAD, RW], f32)
    table_full = nc.dram_tensor("table_full", [NC * NPAD, RW], f32, addr_space="Shared")
    er_loc = nc.dram_tensor("er_loc", [NPAD, ERW], f32)
    hpre = [nc.dram_tensor(f"hpre{l}", [NPAD, D], f32) for l in range(2)]

    with tile.TileContext(nc) as tc:
        with (
            tc.tile_pool(name="const", bufs=1) as cpool,
            tc.tile_pool(name="sb", bufs=2) as sb,
            tc.tile_pool(name="gather", bufs=7) as gp,
            tc.tile_pool(name="spool", bufs=7) as spool,
            tc.tile_pool(name="psA", bufs=2, space="PSUM") as psA,
            tc.tile_pool(name="psB", bufs=1, space="PSUM") as psB,
            tc.tile_pool(name="psW", bufs=3, space="PSUM") as psW,
        ):
            ident = cpool.tile([128, 128], f32)
            make_identity(nc, ident[:])
            gix = cpool.tile([128, SLOTS // 16], i16)
            eix = cpool.tile([128, SLOTS // 16], i16)
            nc.sync.dma_start(out=gix[:], in_=gidx_d[:])
            nc.sync.dma_start(out=eix[:], in_=eidx_d[:])
            W_sb = []
            Wlr_sb = []
            for l in range(2):
                w = cpool.tile([128, 2, D], f32, tag=f"W{l}")
                nc.sync.dma_start(out=w[:], in_=Ws[l][:].rearrange("(k p) n -> p k n", p=128))
                W_sb.append(w)
                wl = cpool.tile([128, 2, 2 * H], f32, tag=f"Wlr{l}")
                nc.sync.dma_start(out=wl[:], in_=Wlrs[l][:].rearrange("(k p) n -> p k n", p=128))
                Wlr_sb.append(wl)

            for layer in range(2):
                # ---------------- phase A: build feature table ----------------
                src_h = hperm if layer == 0 else hpre[0]
                for t in range(NTILES):
                    ht = sb.tile([128, D], f32, tag="ht")
                    nc.sync.dma_start(out=ht[:], in_=src_h[t * 128:(t + 1) * 128, :])
                    if layer == 1:
                        ht = _elu_tile(nc, sb, ht)
                    hT = sb.tile([128, 2, 128], f32, tag="hT")
                    for k in range(2):
                        pt = psA.tile([128, 128], f32, tag="psT")
                        nc.tensor.transpose(pt[:], ht[:, k * 128:(k + 1) * 128], ident[:])
                        nc.vector.tensor_copy(out=hT[:, k, :], in_=pt[:])
                    pf = psA.tile([128, D], f32, tag="psF")
                    plr = psB.tile([128, 2 * H], f32, tag="psLR")
                    for k in range(2):
                        nc.tensor.matmul(
                            pf[:], hT[:, k, :], W_sb[layer][:, k, :],
                            start=(k == 0), stop=(k == 1),
                        )
                    for k in range(2):
                        nc.tensor.matmul(
                            plr[:], hT[:, k, :], Wlr_sb[layer][:, k, :],
                            start=(k == 0), stop=(k == 1),
                        )
                    stg = sb.tile([128, RW], f32, tag="stgA")
                    nc.scalar.copy(out=stg[:, 0:D], in_=pf[:])
                    nc.vector.memset(stg[:, D:D + 1], 1.0)
                    nc.vector.tensor_copy(out=stg[:, D + 1:D + 1 + H], in_=plr[:, 0:H])
                    nc.scalar.dma_start(out=table_loc[t * 128:(t + 1) * 128, :], in_=stg[:])
                    erst = sb.tile([128, ERW], f32, tag="erst")
                    nc.vector.tensor_copy(out=erst[:, 0:H], in_=plr[:, H:2 * H])
                    nc.scalar.dma_start(out=er_loc[t * 128:(t + 1) * 128, :], in_=erst[:])

                nc.gpsimd.collective_compute(
                    "AllGather",
                    mybir.AluOpType.bypass,
                    replica_groups=[list(range(NC))],
                    ins=[table_loc.ap().opt()],
                    outs=[table_full.ap().opt()],
                )

                # ---------------- phase B: edge aggregation ----------------
                for g in range(NGRP):
                    mt = sb.tile([128, CH_PER_G * WSPAN], f32, tag="mask")
                    nc.sync.dma_start(
                        out=mt[:],
                        in_=mask_d[:, g * CH_PER_G * WSPAN:(g + 1) * CH_PER_G * WSPAN],
                    )
                    Gt, Et = [], []
                    for c5 in range(CALLS_PER_G):
                        call = g * CALLS_PER_G + c5
                        lo = c5 < 3
                        gt_ = gp.tile([128, 8, RW], f32, tag="G")
                        src_ap = table_full[0:LO_ROWS, :] if lo else table_full[LO_ROWS:NC * NPAD, :]
                        nc.gpsimd.dma_gather(
                            out_ap=gt_[:],
                            in_ap=src_ap,
                            idxs_ap=gix[:, call * 64:(call + 1) * 64],
                            num_idxs=1024,
                            num_idxs_reg=1024,
                            elem_size=RW,
                            single_packet=True,
                            queue_num=call % 4,
                        )
                        Gt.append(gt_)
                        et_ = gp.tile([128, 8, ERW], f32, tag="E")
                        nc.gpsimd.dma_gather(
                            out_ap=et_[:],
                            in_ap=er_loc[:],
                            idxs_ap=eix[:, call * 64:(call + 1) * 64],
                            num_idxs=1024,
                            num_idxs_reg=1024,
                            elem_size=ERW,
                            single_packet=True,
                            queue_num=(call + 2) % 4,
                        )
                        Et.append(et_)
                    zt = sb.tile([128, CALLS_PER_G, 8, H], f32, tag="z")
                    for c5 in range(CALLS_PER_G):
                        nc.vector.tensor_add(
                            out=zt[:, c5, :, :],
                            in0=Gt[c5][:, :, D + 1:D + 1 + H],
                            in1=Et[c5][:, :, 0:H],
                        )
                    zt2 = sb.tile([128, CALLS_PER_G, 8, H], f32, tag="z2")
                    nc.vector.tensor_scalar_mul(zt2[:], zt[:], NEG_SLOPE)
                    nc.vector.tensor_max(zt[:], zt[:], zt2[:])
                    St = []
                    for c5 in range(CALLS_PER_G):
                        st_ = spool.tile([128, 8, H, WSPAN], f32, tag="S")
                        zb = (
                            zt[:, c5, :, :]
                            .rearrange("p w (h o) -> p w h o", o=1)
                            .to_broadcast([128, 8, H, WSPAN])
                        )
                        mb = (
                            mt[:, c5 * 8 * WSPAN:(c5 + 1) * 8 * WSPAN]
                            .rearrange("p (w o j) -> p w o j", w=8, o=1)
                            .to_broadcast([128, 8, H, WSPAN])
                        )
                        nc.vector.tensor_add(out=st_[:], in0=zb, in1=mb)
                        nc.scalar.activation(st_[:], st_[:], mybir.ActivationFunctionType.Exp)
                        St.append(st_)

                    stg = sb.tile([128, WG, RW - 63], f32, tag="stgB")  # 257 cols
                    for wg in range(WG):
                        pw = psW.tile([128, D + 1], f32, tag="pw")
                        chunks = [
                            (wg * C_LO + k) for k in range(C_LO)
                        ] + [
                            (WG * C_LO + wg * C_HI + k) for k in range(C_HI)
                        ]
                        for i, ch in enumerate(chunks):
                            c5, cic = ch // 8, ch % 8
                            nc.tensor.matmul(
                                pw[:],
                                St[c5][:, cic, :, :].rearrange("p h j -> p (h j)"),
                                Gt[c5][:, cic, 0:D + 1],
                                start=(i == 0),
                                stop=(i == C_W - 1),
                            )
                        dn = sb.tile([128, 1], f32, tag="dn")
                        nc.vector.tensor_scalar_max(dn[:], pw[:, D:D + 1], 1e-30)
                        rc = sb.tile([128, 1], f32, tag="rc")
                        nc.vector.reciprocal(rc[:], dn[:])
                        nc.vector.tensor_scalar_mul(stg[:, wg, :], pw[:], rc[:])
                    dstl = hpre[layer]
                    for h in range(H):
                        nc.scalar.dma_start(
                            out=dstl[g * WG * WSPAN:(g + 1) * WG * WSPAN, h * F:(h + 1) * F]
                            .rearrange("(w j) f -> j w f", j=WSPAN),
                            in_=stg[h * WSPAN:(h + 1) * WSPAN, :, h * F:h * F + F],
                        )

            # ---------------- output: elu(hpre[1]) ----------------
            for t in range(NTILES):
                ht = sb.tile([128, D], f32, tag="ho")
                nc.sync.dma_start(out=ht[:], in_=hpre[1][t * 128:(t + 1) * 128, :])
                o = _elu_tile(nc, sb, ht, tmp_tag="eluo")
                nc.scalar.dma_start(out=out_d[t * 128:(t + 1) * 128, :], in_=o[:])

    nc.compile()
    _PROGRAM_CACHE[key] = nc
    return nc


# =====================================================================
# entry point
# =====================================================================

def _host_wlr(W, al, ar):
    Wr = W.reshape(D, H, F)
    wl = np.einsum("dhf,hf->dh", Wr, al)
    wr = np.einsum("dhf,hf->dh", Wr, ar)
    return np.concatenate([wl, wr], axis=1).astype(np.float32)  # [D, 2H]


def kernel(h, src, dst, W1, al1, ar1, b1, W2, al2, ar2, b2):
    assert not np.any(b1) and not np.any(b2), "nonzero bias not supported"
    nc = build_program()
    cores = preprocess(np.asarray(src), np.asarray(dst))
    Wlr1 = _host_wlr(W1, al1, ar1)
    Wlr2 = _host_wlr(W2, al2, ar2)
    in_maps = []
    for c in range(NC):
        cc = cores[c]
        hp = np.zeros((NPAD, D), np.float32)
        valid = cc["perm"] >= 0
        hp[valid] = h[c * NLOC + cc["perm"][valid]]
        in_maps.append(
            dict(
                hperm=hp,
                W1=np.ascontiguousarray(W1, np.float32),
                W2=np.ascontiguousarray(W2, np.float32),
                Wlr1=Wlr1,
                Wlr2=Wlr2,
                gidx=cc["gidx"],
                eidx=cc["eidx"],
                maskd=cc["mask"].reshape(128, NCH * WSPAN),
            )
        )
    res = run_bass_kernel_spmd(nc, in_maps, core_ids=list(range(NC)))
    out = np.empty((N, D), np.float32)
    for c in range(NC):
        o = res.results[c]["out"]
        pos = cores[c]["perm_pos"]
        out[c * NLOC:(c + 1) * NLOC] = o[pos]
    return out


if __name__ == "__main__":
    rng = np.random.default_rng(0)
    h = rng.normal(size=(N, D)).astype(np.float32)
    src = rng.integers(0, N, size=E).astype(np.int32)
    dst = rng.integers(0, N, size=E).astype(np.int32)
    W1 = (rng.normal(size=(D, D)) * 0.05).astype(np.float32)
    al1 = (rng.normal(size=(H, F)) * 0.05).astype(np.float32)
    ar1 = (rng.normal(size=(H, F)) * 0.05).astype(np.float32)
    b1 = np.zeros(D, np.float32)
    out = kernel(h=h, src=src, dst=dst, W1=W1, al1=al1, ar1=ar1, b1=b1,
                 W2=W1, al2=al1, ar2=ar1, b2=b1)
    print("out", out.shape, out[:2, :4])


# revision 9
# speedup vs baseline: 2.4405x; 2.4405x over previous
"""GAT (2-layer, 4-head) on 8 Trainium2 NeuronCores.

Strategy (dst-sharded, per sharding hint):
  - Nodes partitioned into 8 contiguous blocks of 6250; core c owns block c
    and all edges whose dst lies in it.
  - Host bin-packs each core's dst nodes into 200 "windows" of <=32 dsts,
    bounding per-window lo/hi edge counts (lo = src owner < 5, needed because
    dma_gather indices are signed int16).  Nodes are stored in permuted
    (window) order on device; the host un-permutes the final output.
  - Per layer: feat = h @ [W | Wl | Wr] computed locally (PE, bf16), feature
    table AllGather'ed, then per-edge rows gathered with dma_gather (bf16,
    768B rows); edge softmax + weighted segment-sum are done as one-hot
    matmuls on the PE with masks precomputed on host. er(dst) enters through
    a per-group broadcast added into the mask.
"""

import sys

sys.path.insert(0, "/opt/trn_rl_repo")

import numpy as np

import concourse.bacc as bacc
import concourse.mybir as mybir
import concourse.tile as tile
from concourse import bass
from concourse.bass_utils import run_bass_kernel_spmd
from concourse.masks import make_identity

# ---------------- problem constants ----------------
N = 50000
E = 800000
D = 256           # in feats = H*F
H = 4
F = 64
NC = 8
NLOC = N // NC    # 6250
NEG_SLOPE = 0.2

# ---------------- sharding constants ----------------
WSPAN = 32        # dsts per window
NWIN = 200        # windows per core
C_LO = 3          # lo chunks per window (<=384 lo edges)
C_HI = 2          # hi chunks per window (<=256 hi edges)
C_W = C_LO + C_HI
WG = 8            # windows per group
NGRP = NWIN // WG          # 25
CH_PER_G = WG * C_W        # 40 chunks per group
CALLS_PER_G = CH_PER_G // 8  # 5 gather calls per group (1024 idx each)
NCH = NWIN * C_W           # 1000 chunks per layer
SLOTS = NCH * 128          # 128000 edge slots
NPAD = NWIN * WSPAN        # 6400 padded rows per core
RW = 384          # table row width (bf16): 256 feat | 1 one | 4 el | 123 pad
WGT = D + 2 * H   # fused weight cols: 256 feat + 4 el + 4 er
LO_ROWS = 5 * NPAD          # 32000 (int16-safe)
NTILES = NPAD // 128        # 50
NEG_BIG = -1.0e30

_PROGRAM_CACHE = {}


# =====================================================================
# host-side graph preprocessing
# =====================================================================

def _preprocess_core(src_c, dst_loc_c):
    lo_e = src_c < 5 * NLOC
    nlo = np.bincount(dst_loc_c[lo_e], minlength=NLOC)
    nhi = np.bincount(dst_loc_c[~lo_e], minlength=NLOC)
    order = np.argsort(-(nlo + nhi), kind="stable")
    cap_lo, cap_hi = C_LO * 128, C_HI * 128
    win_lo = np.zeros(NWIN, np.int64)
    win_hi = np.zeros(NWIN, np.int64)
    win_cnt = np.zeros(NWIN, np.int64)
    node_win = np.full(NLOC, -1, np.int64)
    node_slot = np.full(NLOC, -1, np.int64)
    w_hint = 0
    for u in order:
        placed = False
        for off in range(NWIN):
            w = (w_hint + off) % NWIN
            if (
                win_cnt[w] < WSPAN
                and win_lo[w] + nlo[u] <= cap_lo
                and win_hi[w] + nhi[u] <= cap_hi
            ):
                node_win[u] = w
                node_slot[u] = win_cnt[w]
                win_cnt[w] += 1
                win_lo[w] += nlo[u]
                win_hi[w] += nhi[u]
                w_hint = (w + 1) % NWIN
                placed = True
                break
        if not placed:
            raise RuntimeError("window bin-packing failed")

    perm = np.full(NPAD, -1, np.int64)
    perm_pos = node_win * WSPAN + node_slot
    perm[perm_pos] = np.arange(NLOC)

    e_win = node_win[dst_loc_c]
    e_lo = lo_e.astype(np.int64)
    eorder = np.lexsort((1 - e_lo, e_win))
    ew = e_win[eorder]
    el_ = e_lo[eorder]
    esrc = src_c[eorder]
    edst = dst_loc_c[eorder]

    eidx_flat = np.zeros(SLOTS, np.int64)
    mask = np.full((128, NCH, WSPAN), NEG_BIG, np.float32)

    win_edge_start = np.zeros(NWIN + 1, np.int64)
    np.add.at(win_edge_start, ew + 1, 1)
    win_edge_start = np.cumsum(win_edge_start)

    # vectorized slot assignment (lo edges first within each window)
    e_arange = np.arange(len(esrc))
    e_ofs = e_arange - win_edge_start[ew]
    nl_win = np.zeros(NWIN, np.int64)
    np.add.at(nl_win, ew, el_)
    is_lo = el_ == 1
    lo_i = e_ofs
    hi_i = e_ofs - nl_win[ew]
    g_, wg_ = ew // WG, ew % WG
    ch = np.where(
        is_lo,
        g_ * CH_PER_G + wg_ * C_LO + lo_i // 128,
        g_ * CH_PER_G + WG * C_LO + wg_ * C_HI + hi_i // 128,
    )
    p_ = np.where(is_lo, lo_i % 128, hi_i % 128)
    slot_of_edge = ch * 128 + p_
    jcol = node_slot[edst]
    mask[p_, ch, jcol] = 0.0
    eidx_flat[slot_of_edge] = ew * WSPAN + jcol

    return dict(
        perm=perm,
        perm_pos=perm_pos,
        slot_of_edge=slot_of_edge,
        esrc=esrc,
        e_lo=el_,
        mask=mask,
        eidx_flat=eidx_flat,
    )


def _wrap_idx(flat_i64):
    a = flat_i64.astype(np.int16)
    blk = a.reshape(SLOTS // 16, 16).T
    return np.tile(blk, (8, 1)).copy()


def preprocess(src, dst):
    dst_owner = dst // NLOC
    cores = []
    for c in range(NC):
        m = dst_owner == c
        cores.append(_preprocess_core(src[m], dst[m] - c * NLOC))
    perm_pos_all = np.stack([cores[c]["perm_pos"] for c in range(NC)])
    for c in range(NC):
        cc = cores[c]
        owner = cc["esrc"] // NLOC
        src_loc = cc["esrc"] - owner * NLOC
        gid = owner * NPAD + perm_pos_all[owner, src_loc]
        gl = np.zeros(SLOTS, np.int64)
        gl[cc["slot_of_edge"]] = np.where(cc["e_lo"] == 1, gid, gid - LO_ROWS)
        assert gl.max() < 32768 and gl.min() >= 0
        cc["gidx"] = _wrap_idx(gl)
    return cores


# =====================================================================
# device program
# =====================================================================

def _elu_tile(nc, pool, x, tmp_tag="elu"):
    f32 = mybir.dt.float32
    m = pool.tile([128, D], f32, tag=tmp_tag + "_m")
    q = pool.tile([128, D], f32, tag=tmp_tag + "_q")
    o = pool.tile([128, D], f32, tag=tmp_tag + "_o")
    nc.vector.tensor_scalar_min(m[:], x[:], 0.0)
    nc.scalar.activation(q[:], m[:], mybir.ActivationFunctionType.Exp)
    nc.vector.tensor_sub(o[:], x[:], m[:])
    nc.vector.tensor_add(o[:], o[:], q[:])
    nc.vector.tensor_scalar_add(o[:], o[:], -1.0)
    return o


def build_program():
    key = "main"
    if key in _PROGRAM_CACHE:
        return _PROGRAM_CACHE[key]
    nc = bacc.Bacc(
        "TRN2",
        target_bir_lowering=False,
        debug=False,
        num_devices=NC,
        num_swdge_queues=4,
    )
    f32 = mybir.dt.float32
    bf16 = mybir.dt.bfloat16
    i16 = mybir.dt.int16

    hperm = nc.dram_tensor("hperm", [NPAD, D], f32, kind="ExternalInput")
    Wc = [nc.dram_tensor(f"Wc{i}", [D, WGT], bf16, kind="ExternalInput") for i in (1, 2)]
    gidx_d = nc.dram_tensor("gidx", [128, SLOTS // 16], i16, kind="ExternalInput")
    mask_d = nc.dram_tensor("maskd", [128, NCH * WSPAN], bf16, kind="ExternalInput")
    out_d = nc.dram_tensor("out", [NPAD, D], f32, kind="ExternalOutput")

    table_loc = nc.dram_tensor("table_loc", [NPAD, RW], bf16)
    table_full = nc.dram_tensor("table_full", [NC * NPAD, RW], bf16, addr_space="Shared")
    er_loc = nc.dram_tensor("er_loc", [NWIN, H, WSPAN], bf16)  # er^T per window
    hpre = [nc.dram_tensor(f"hpre{l}", [NPAD, D], f32) for l in range(2)]

    with tile.TileContext(nc) as tc:
        with (
            tc.tile_pool(name="const", bufs=1) as cpool,
            tc.tile_pool(name="sb", bufs=2) as sb,
            tc.tile_pool(name="gather", bufs=10) as gp,
            tc.tile_pool(name="spool", bufs=10) as spool,
            tc.tile_pool(name="psA", bufs=2, space="PSUM") as psA,
            tc.tile_pool(name="psW", bufs=4, space="PSUM") as psW,
        ):
            ident = cpool.tile([128, 128], f32)
            make_identity(nc, ident[:])
            gix = cpool.tile([128, SLOTS // 16], i16)
            nc.sync.dma_start(out=gix[:], in_=gidx_d[:])
            W_sb = []
            for l in range(2):
                w = cpool.tile([128, 2, WGT], bf16, tag=f"W{l}")
                nc.sync.dma_start(out=w[:], in_=Wc[l][:].rearrange("(k p) n -> p k n", p=128))
                W_sb.append(w)

            for layer in range(2):
                # ---------------- phase A: build feature table ----------------
                src_h = hperm if layer == 0 else hpre[0]
                for t in range(NTILES):
                    ht = sb.tile([128, D], f32, tag="ht")
                    nc.sync.dma_start(out=ht[:], in_=src_h[t * 128:(t + 1) * 128, :])
                    if layer == 1:
                        ht = _elu_tile(nc, sb, ht)
                    hT = sb.tile([128, 2, 128], bf16, tag="hT")
                    for k in range(2):
                        pt = psA.tile([128, 128], f32, tag="psT")
                        nc.tensor.transpose(pt[:], ht[:, k * 128:(k + 1) * 128], ident[:])
                        nc.vector.tensor_copy(out=hT[:, k, :], in_=pt[:])
                    pf = psA.tile([128, WGT], f32, tag="psF")
                    for k in range(2):
                        nc.tensor.matmul(
                            pf[:], hT[:, k, :], W_sb[layer][:, k, :],
                            start=(k == 0), stop=(k == 1),
                        )
                    stg = sb.tile([128, RW], bf16, tag="stgA")
                    nc.scalar.copy(out=stg[:, 0:D], in_=pf[:, 0:D])
                    nc.vector.memset(stg[:, D:D + 1], 1.0)
                    nc.vector.tensor_copy(out=stg[:, D + 1:D + 1 + H], in_=pf[:, D:D + H])
                    nc.sync.dma_start(out=table_loc[t * 128:(t + 1) * 128, :], in_=stg[:])
                    erst = sb.tile([128, H], f32, tag="erst")
                    nc.vector.tensor_copy(out=erst[:], in_=pf[:, D + H:D + 2 * H])
                    ptE = psA.tile([128, 128], f32, tag="psT")
                    nc.tensor.transpose(ptE[0:H, :], erst[:], ident[:])
                    erT = sb.tile([H, 128], bf16, tag="erT")
                    nc.vector.tensor_copy(out=erT[:], in_=ptE[0:H, :])
                    nc.sync.dma_start(
                        out=er_loc[t * 4:(t + 1) * 4, :, :].rearrange("w h j -> h w j"),
                        in_=erT[:].rearrange("h (w j) -> h w j", j=WSPAN),
                    )

                nc.gpsimd.collective_compute(
                    "AllGather",
                    mybir.AluOpType.bypass,
                    replica_groups=[list(range(NC))],
                    ins=[table_loc.ap().opt()],
                    outs=[table_full.ap().opt()],
                )

                # ---------------- phase B: edge aggregation ----------------
                for g in range(NGRP):
                    mt = sb.tile([128, CH_PER_G * WSPAN], bf16, tag="mask")
                    nc.sync.dma_start(
                        out=mt[:],
                        in_=mask_d[:, g * CH_PER_G * WSPAN:(g + 1) * CH_PER_G * WSPAN],
                    )
                    # er values of this group's 256 dsts broadcast to all
                    # partitions: er_rep[p, w, h, j]
                    er_rep = sb.tile([128, WG, H, WSPAN], bf16, tag="errep")
                    nc.sync.dma_start(
                        out=er_rep[:],
                        in_=er_loc[g * WG:(g + 1) * WG, :, :]
                        .rearrange("(o w) h j -> o (w h j)", o=1)
                        .to_broadcast([128, WG * H * WSPAN])
                        .rearrange("p (w h j) -> p w h j", w=WG, h=H),
                    )
                    # maskER[p, ch, h, j] = mask[p, ch, j] + er[w(ch), h, j]
                    mer = sb.tile([128, CH_PER_G, H, WSPAN], bf16, tag="mer")
                    for wg in range(WG):
                        lo0 = wg * C_LO
                        nc.vector.tensor_add(
                            out=mer[:, lo0:lo0 + C_LO, :, :],
                            in0=mt[:, lo0 * WSPAN:(lo0 + C_LO) * WSPAN]
                            .rearrange("p (c o j) -> p c o j", c=C_LO, o=1)
                            .to_broadcast([128, C_LO, H, WSPAN]),
                            in1=er_rep[:, wg, :, :]
                            .rearrange("p (o h) j -> p o h j", o=1)
                            .to_broadcast([128, C_LO, H, WSPAN]),
                        )
                        hi0 = WG * C_LO + wg * C_HI
                        nc.vector.tensor_add(
                            out=mer[:, hi0:hi0 + C_HI, :, :],
                            in0=mt[:, hi0 * WSPAN:(hi0 + C_HI) * WSPAN]
                            .rearrange("p (c o j) -> p c o j", c=C_HI, o=1)
                            .to_broadcast([128, C_HI, H, WSPAN]),
                            in1=er_rep[:, wg, :, :]
                            .rearrange("p (o h) j -> p o h j", o=1)
                            .to_broadcast([128, C_HI, H, WSPAN]),
                        )
                    Gt = []
                    for c5 in range(CALLS_PER_G):
                        call = g * CALLS_PER_G + c5
                        lo = c5 < 3
                        gt_ = gp.tile([128, 8, RW], bf16, tag="G")
                        src_ap = table_full[0:LO_ROWS, :] if lo else table_full[LO_ROWS:NC * NPAD, :]
                        nc.gpsimd.dma_gather(
                            out_ap=gt_[:],
                            in_ap=src_ap,
                            idxs_ap=gix[:, call * 64:(call + 1) * 64],
                            num_idxs=1024,
                            num_idxs_reg=1024,
                            elem_size=RW,
                            single_packet=True,
                            queue_num=call % 4,
                        )
                        Gt.append(gt_)
                    St = []
                    for c5 in range(CALLS_PER_G):
                        st_ = spool.tile([128, 8, H, WSPAN], bf16, tag="S")
                        u_ = spool.tile([128, 8, H, WSPAN], bf16, tag="U")
                        elb = (
                            Gt[c5][:, :, D + 1:D + 1 + H]
                            .rearrange("p w (h o) -> p w h o", o=1)
                            .to_broadcast([128, 8, H, WSPAN])
                        )
                        nc.vector.tensor_add(
                            out=st_[:], in0=elb,
                            in1=mer[:, c5 * 8:(c5 + 1) * 8, :, :],
                        )
                        nc.vector.tensor_scalar_mul(u_[:], st_[:], NEG_SLOPE)
                        nc.vector.tensor_max(st_[:], st_[:], u_[:])
                        nc.scalar.activation(st_[:], st_[:], mybir.ActivationFunctionType.Exp)
                        St.append(st_)

                    stg = sb.tile([128, WG, D + 1], f32, tag="stgB")
                    for wg in range(WG):
                        pw = psW.tile([128, D + 1], f32, tag="pw")
                        chunks = [wg * C_LO + k for k in range(C_LO)] + [
                            WG * C_LO + wg * C_HI + k for k in range(C_HI)]
                        for i, ch in enumerate(chunks):
                            c5, cic = ch // 8, ch % 8
                            nc.tensor.matmul(
                                pw[:],
                                St[c5][:, cic, :, :].rearrange("p h j -> p (h j)"),
                                Gt[c5][:, cic, 0:D + 1],
                                start=(i == 0),
                                stop=(i == C_W - 1),
                            )
                        dn = sb.tile([128, 1], f32, tag="dn")
                        nc.vector.tensor_scalar_max(dn[:], pw[:, D:D + 1], 1e-30)
                        rc = sb.tile([128, 1], f32, tag="rc")
                        nc.vector.reciprocal(rc[:], dn[:])
                        nc.vector.tensor_scalar_mul(stg[:, wg, :], pw[:], rc[:])
                    dstl = hpre[layer]
                    for h in range(H):
                        nc.sync.dma_start(
                            out=dstl[g * WG * WSPAN:(g + 1) * WG * WSPAN, h * F:(h + 1) * F]
                            .rearrange("(w j) f -> j w f", j=WSPAN),
                            in_=stg[h * WSPAN:(h + 1) * WSPAN, :, h * F:h * F + F],
                        )

            # ---------------- output: elu(hpre[1]) ----------------
            for t in range(NTILES):
                ht = sb.tile([128, D], f32, tag="ho")
                nc.sync.dma_start(out=ht[:], in_=hpre[1][t * 128:(t + 1) * 128, :])
                o = _elu_tile(nc, sb, ht, tmp_tag="eluo")
                nc.sync.dma_start(out=out_d[t * 128:(t + 1) * 128, :], in_=o[:])

    nc.compile()
    _PROGRAM_CACHE[key] = nc
    return nc


# =====================================================================
# entry point
# =====================================================================

def _host_wc(W, al, ar):
    Wr = W.reshape(D, H, F)
    wl = np.einsum("dhf,hf->dh", Wr, al)
    wr = np.einsum("dhf,hf->dh", Wr, ar)
    return np.concatenate([W, wl, wr], axis=1).astype(np.float32)


def _to_bf16(x):
    import ml_dtypes
    return np.asarray(x).astype(ml_dtypes.bfloat16)


def kernel(h, src, dst, W1, al1, ar1, b1, W2, al2, ar2, b2):
    assert not np.any(b1) and not np.any(b2), "nonzero bias not supported"
    nc = build_program()
    cores = preprocess(np.asarray(src), np.asarray(dst))
    Wc1 = _to_bf16(_host_wc(W1, al1, ar1))
    Wc2 = _to_bf16(_host_wc(W2, al2, ar2))
    in_maps = []
    for c in range(NC):
        cc = cores[c]
        hp = np.zeros((NPAD, D), np.float32)
        valid = cc["perm"] >= 0
        hp[valid] = h[c * NLOC + cc["perm"][valid]]
        in_maps.append(
            dict(
                hperm=hp,
                Wc1=Wc1,
                Wc2=Wc2,
                gidx=cc["gidx"],
                maskd=_to_bf16(cc["mask"].reshape(128, NCH * WSPAN)),
            )
        )
    res = run_bass_kernel_spmd(nc, in_maps, core_ids=list(range(NC)))
    out = np.empty((N, D), np.float32)
    for c in range(NC):
        o = res.results[c]["out"]
        pos = cores[c]["perm_pos"]
        out[c * NLOC:(c + 1) * NLOC] = o[pos]
    return out


if __name__ == "__main__":
    rng = np.random.default_rng(0)
    h = rng.normal(size=(N, D)).astype(np.float32)
    src = rng.integers(0, N, size=E).astype(np.int32)
    dst = rng.integers(0, N, size=E).astype(np.int32)
    W1 = (rng.normal(size=(D, D)) * 0.05).astype(np.float32)
    al1 = (rng.normal(size=(H, F)) * 0.05).astype(np.float32)
    ar1 = (rng.normal(size=(H, F)) * 0.05).astype(np.float32)
    b1 = np.zeros(D, np.float32)
    out = kernel(h=h, src=src, dst=dst, W1=W1, al1=al1, ar1=ar1, b1=b1,
                 W2=W1, al2=al1, ar2=ar1, b2=b1)
    print("out", out.shape, out[:2, :4])
